# revision 34
# baseline (speedup 1.0000x reference)
"""Trainium2 Bass kernel for a 4-layer binarized MLP (BNN) in eval mode.

Network (B=16384, D_in=784, H=4096, D_out=10), all matmuls use sign(w):
  h1 = hardtanh(BN1(x @ sign(w1).T + b1))
  h2 = hardtanh(BN2(sign(h1) @ sign(w2).T + b2))
  h3 = hardtanh(BN3(sign(h2) @ sign(w3).T + b3))
  out = log_softmax(sign(h3) @ sign(w4).T + b4)

Key observations used here:
  * Only the SIGN of h1/h2/h3 matters downstream (hardtanh preserves sign),
    so each hidden layer reduces to  a_out = Sign(psum * s + c)  with
    s = g*rsqrt(v+eps), c = (b-m)*s + be  folded on the host.
  * sign() values are exactly representable in fp8e4/bf16, and matmuls of
    +-1 values accumulate exactly in fp32 PSUM -> layers 2..4 are exact.
  * Layer 1 needs ~fp32 precision on x: x is split into two fp16 terms
    (hi+lo captures ~22 mantissa bits; PE handles fp16 subnormals exactly).
    Both terms are CONCATENATED along K (with sign(w1).T stacked twice) so
    layer 1 is one [1664, B] x [1664, 4096] matmul accumulated in PSUM.
  * Data-parallel over 8 cores: batch is sharded 8 x 2048; weights are
    binarized+transposed+pre-tiled on the host and replicated.

Layout on device (feature-major activations; batch on the free dim):
  activations a_l : SBUF [128, 32 ktiles, B_CHUNK]   (fp8e4 +-1)
  weights W_l^T   : DRAM [32 jtiles, 128 kp, ktiles, 128 cols], streamed
                    per j-tile; matmul lhsT = wt[:, kt, :]  (stationary)
  psum            : [128, B_CHUNK] fp32, accumulated over ktiles
Final layer produces y4 [16, B] (10 valid rows), PE-transposed in 128-col
chunks into one PSUM tile, then log_softmax along the free dim and DMA to
the output [B_shard, 10].

Schedule notes (measured on HW; the kernel runs at ~97% PE stream
occupancy, which is the roofline for this decomposition -- fp16 streams 1
moving col/cycle and fp8 DoubleRow streams 2, so every 512-col matmul
instruction costs ~214ns regardless of layer; 5840 such instructions =
~1246us of irreducible stream time at the fp8 silicon peak):
  * PE warm-up: ~18 junk matmuls over a memset tile run while the first
    x/w DMAs stream, so the HAM clock-gate reaches 2.4 GHz before real
    work and the in-order PE queue never idles long enough to re-throttle.
  * Chunk 0's L1 runs in two 512-col sweeps so the startup only waits on
    half the xt bytes (the ramp is DMA-bandwidth-bound); the second xt
    half and sweep B's w1 re-stream hide inside sweep A.
  * L3+L4 run in 512-col batch halves; each half's L4 psum completes at
    that half's end, so every softmax tail hides inside the next half's
    (or next chunk's L1) matmul stream.  Only the last half-tail is
    exposed (~3.5us), pipelined in two quarters.
  * All activations are pinned to the single ACT table set that contains
    sign+exp+ln+identity ("natural_log_exp_and_others"); otherwise the
    hardware reloads 1.3us activation tables on every exp<->ln switch.
  * Softmax per half: 2 ACT ops (one EXP, one LN); per-group max/bias
    arithmetic runs on the otherwise-idle DVE via per-partition-scalar
    and broadcast ops; output DMAs rotate across the trigger queues.
  * Known dead ends (measured): fp8 stationary x fp16 moving runs ~2x
    slower per column (keep w1 fp16); 512-col chunks double the per-phase
    weight-DMA rate and starve the single weights queue; DoubleColumn/
    DoublePixel are uint8-only and matmul_mx is TRN3-only, so 2 MAC/cell/
    cycle (DoubleRow) is the TRN2 fp8 ceiling.
"""

import numpy as np
import ml_dtypes

# ---- problem constants (hardcoded per the harness contract) ----
B, D_IN, H, OUT = 16384, 784, 4096, 10
N_CORES = 8
BS = B // N_CORES          # 2048 rows per core
# batch chunks per core (chunk 0's L1 additionally runs in two 512-col
# sweeps so the DMA-bandwidth-bound ramp only waits on half the xt bytes)
CHUNKS = (1024, 1024)
BCX = max(CHUNKS)          # tile allocation size (chunks use a [:bc] slice)
KD = 13                    # 1664 = 13*128 k-tiles: [hi;lo] fp16 concat (2*784 padded)
KH = H // 128              # 32 k-tiles for hidden layers
JT = H // 128              # 32 output-channel tiles
N0 = 512                   # matmul moving free-dim chunk
OP = 16                    # padded output channels (10 -> 16)
BN_EPS = 1e-5

_BF16 = ml_dtypes.bfloat16

_compiled = None  # cache of (nc, run_fn)


def _build_module():
    import concourse.bass as bass
    import concourse.tile as tile
    from concourse import bacc, mybir
    from concourse.masks import make_identity
    from contextlib import ExitStack

    f32 = mybir.dt.float32
    bf16 = mybir.dt.bfloat16
    fp16 = mybir.dt.float16
    fp8 = mybir.dt.float8e4
    AF = mybir.ActivationFunctionType
    AX = mybir.AxisListType

    nc = bacc.Bacc("TRN2", target_bir_lowering=False, debug=False,
                   num_devices=N_CORES)

    dr = {}
    dr["xt"] = nc.dram_tensor("xt", [128, KD, BS], fp16, kind="ExternalInput").ap()
    dr["w1"] = nc.dram_tensor("w1", [JT, 128, KD, 128], fp16, kind="ExternalInput").ap()
    dr["w2"] = nc.dram_tensor("w2", [JT, 128, KH, 128], fp8, kind="ExternalInput").ap()
    dr["w3"] = nc.dram_tensor("w3", [JT, 128, KH, 128], fp8, kind="ExternalInput").ap()
    dr["w4"] = nc.dram_tensor("w4", [128, KH, OP], fp8, kind="ExternalInput").ap()
    # all six BN fold tensors packed into one DMA: [128, layer, {s,c}, JT]
    dr["bnc"] = nc.dram_tensor("bnc", [128, 3, 2, JT], f32, kind="ExternalInput").ap()
    dr["b4"] = nc.dram_tensor("b4", [OP, 1], f32, kind="ExternalInput").ap()
    out_dram = nc.dram_tensor("out", [BS, OUT], f32, kind="ExternalOutput").ap()

    with tile.TileContext(nc) as tc, ExitStack() as ctx:
        const = ctx.enter_context(tc.tile_pool(name="const", bufs=1))
        xt_pool = ctx.enter_context(tc.tile_pool(name="xt", bufs=1))
        # deep prefetch: several w1 j-tiles in flight during the ramp (a
        # late w1 DMA stalls its whole j-tile and lets the HAM clock-gate
        # re-throttle the PE)
        w1pool = ctx.enter_context(tc.tile_pool(name="w1p", bufs=6))
        wpool = ctx.enter_context(tc.tile_pool(name="wp", bufs=3))
        apool = ctx.enter_context(tc.tile_pool(name="acts", bufs=2))
        small = ctx.enter_context(tc.tile_pool(name="small", bufs=8))
        pspool = ctx.enter_context(tc.tile_pool(name="ps", bufs=2, space="PSUM"))
        ps4pool = ctx.enter_context(tc.tile_pool(name="ps4", bufs=1, space="PSUM"))
        pstpool = ctx.enter_context(tc.tile_pool(name="pst", bufs=2, space="PSUM"))

        # ---- PE warm-up: the HAM clock gate keeps the PE at 1.2 GHz until
        # it has been busy ~3.4us.  Real matmuls can't start until the first
        # x/w DMAs land (~2.5us after the engine barrier), so burn that idle
        # window on junk matmuls over a memset tile: the PE is then already
        # at 2.4 GHz when the real stream begins. ----
        junk = const.tile([128, 640], fp16, tag="junk")
        nc.gpsimd.memset(junk, 0.0)
        jps = pspool.tile([128, BCX], f32, tag="ps", name="warmps")
        for _ in range(18):
            nc.tensor.matmul(jps[:, 0:N0], junk[:, 0:128],
                             junk[:, 128:128 + N0], start=True, stop=True)

        def load_xt(b_off, c0, c1, t=None):
            # one SBUF tile, 2-ktile DMA slices alternating across the two
            # non-weight trigger queues (sync carries weights; only
            # gpsimd/sync/scalar can trigger DMAs).  Fewer descriptors =
            # more ramp bandwidth; slice-level deps keep the early matmuls
            # gated only on their own k-tiles.  kt0 ships alone so the
            # very first matmul starts as soon as possible.  [c0, c1) is
            # the column range loaded (the startup loads cols in halves,
            # passing the same tile handle so slice deps line up).
            qs = (nc.gpsimd, nc.scalar)
            if t is None:
                t = xt_pool.tile([128, KD, BCX], fp16, tag="xt", name="xtt")
            nc.gpsimd.dma_start(out=t[:, 0, c0:c1],
                                in_=dr["xt"][:, 0, b_off + c0:b_off + c1])
            for i, kt in enumerate(range(1, KD, 2)):
                ke = min(kt + 2, KD)
                qs[(i + 1) % 2].dma_start(
                    out=t[:, kt:ke, c0:c1],
                    in_=dr["xt"][:, kt:ke, b_off + c0:b_off + c1])
            return t

        # chunk 0's L1 runs in two 512-col sweeps, so only half the xt
        # bytes gate the ramp; the second half streams during sweep A
        xt0_t = load_xt(0, 0, N0)
        xt0 = [xt0_t[:, kt, :] for kt in range(KD)]

        # constants (single packed DMA; needed only by epilogues)
        bnc = const.tile([128, 3, 2, JT], f32, tag="bnc")
        nc.scalar.dma_start(out=bnc, in_=dr["bnc"])
        s_sb = {l: bnc[:, l - 1, 0, :] for l in (1, 2, 3)}
        c_sb = {l: bnc[:, l - 1, 1, :] for l in (1, 2, 3)}
        b4sb = const.tile([OP, 1], f32, tag="b4")
        nc.scalar.dma_start(out=b4sb, in_=dr["b4"])
        w4t = const.tile([128, KH, OP], fp8, tag="w4")
        nc.scalar.dma_start(out=w4t, in_=dr["w4"])
        # identity for the PE transposes: materialized lazily (first use is
        # the first chunk's softmax tail, deep into chunk 1's L1 j-loop) so
        # its gpsimd ops don't sit in the startup critical path
        ident = const.tile([OP, OP], f32, tag="ident")

        def emit_tail_y4(ps4_t, bc):
            """y4 = ps4 + b4, in halves on the idle DVE so it overlaps the
            last L3 epilogue (ACT) and transposes start sooner."""
            y4 = small.tile([OP, BCX], f32, tag="y4")
            for h in range(0, bc, bc // 2):
                nc.vector.tensor_scalar_add(y4[:, h:h + bc // 2],
                                            ps4_t[:, h:h + bc // 2],
                                            b4sb[:, 0:1])
            return y4

        def emit_tail_softmax(y4, b_off, bc, halves=1):
            """log_softmax with 2 ACT ops per half: one EXP, one LN.

            Per-group max-subtract / sums / final bias-add run on the idle
            DVE as per-partition-scalar ops, so the ACT engine never
            ping-pongs activation tables mid-chain.  halves=2 pipelines the
            chain in two batch halves (used for the final, fully exposed
            tail: half 1's ACT/DVE chain overlaps half 2's transposes).
            """
            from concourse.bass import broadcast_tensor_aps
            G = bc // 128
            Gh = G // halves
            GX = BCX // 128
            for h in range(halves):
                g0 = h * Gh
                pstf = pstpool.tile([128, GX, OP], f32, tag="pst", name="pstf")
                pst = pstf[:, 0:Gh, :]
                for t in range(Gh):
                    gt = g0 + t
                    nc.tensor.transpose(pst[:, t, :],
                                        y4[:, gt * 128:(gt + 1) * 128], ident)
                mx8 = small.tile([128, GX, 1], f32, tag="mx8", name="mx8")[:, 0:Gh, :]
                nc.vector.reduce_max(mx8[:, :, 0], pst[:, :, 0:OUT],
                                     axis=AX.X, negate=True)
                xs = small.tile([128, GX, OP], f32, tag="xs", name="xs")[:, 0:Gh, :]
                p_bc, m_bc = broadcast_tensor_aps(pst[:, :, :], mx8[:, :, :])
                nc.vector.tensor_add(xs, p_bc, m_bc)
                ex = small.tile([128, GX, OP], f32, tag="ex", name="ex")[:, 0:Gh, :]
                nc.scalar.activation(ex, xs, AF.Exp)
                sm8 = small.tile([128, GX, 1], f32, tag="sm8", name="sm8")[:, 0:Gh, :]
                nc.vector.reduce_sum(sm8[:, :, 0], ex[:, :, 0:OUT], axis=AX.X)
                lg8 = small.tile([128, GX, 1], f32, tag="lg8", name="lg8")[:, 0:Gh, :]
                nc.scalar.activation(lg8, sm8, AF.Ln)
                b28 = small.tile([128, GX, 1], f32, tag="b28", name="b28")[:, 0:Gh, :]
                nc.vector.tensor_sub(b28, mx8, lg8)
                oo = small.tile([128, GX, OP], f32, tag="oo", name="oo")[:, 0:Gh, :]
                p_bc2, b_bc = broadcast_tensor_aps(pst[:, :, :], b28[:, :, :])
                nc.vector.tensor_add(oo, p_bc2, b_bc)
                qs = (nc.sync, nc.gpsimd, nc.scalar)
                for t in range(Gh):
                    row0 = b_off + (g0 + t) * 128
                    qs[t % 3].dma_start(out=out_dram[row0:row0 + 128, :],
                                        in_=oo[:, t, 0:OUT])

        pending_tail = None  # (ps4_tile, b_off, bc) of the previous chunk

        b_off = 0
        for cb, bc in enumerate(CHUNKS):
            if cb == 0:
                xt = xt0
            else:
                xtt = load_xt(b_off, 0, bc)
                xt = [xtt[:, kt, :] for kt in range(KD)]

            # ---- layer 1: [hi;lo] fp16 concat matmul.  Chunk 0 runs in
            # two 512-col sweeps (w1 streamed once per sweep, spread over
            # the queues) so the ramp only waits on half the xt bytes ----
            a1 = apool.tile([128, KH, BCX], fp8, tag="act")
            sweeps = [(0, N0), (N0, N0)] if cb == 0 else [(0, bc)]
            for sw, (s0, sn) in enumerate(sweeps):
                for j in range(JT):
                    wt = w1pool.tile([128, KD, 128], fp16, tag="w1")
                    if cb == 0 and sw == 0 and j == 0:
                        # first k-tile lands first so matmul 0 starts
                        # sooner; the rest in two pieces so the j0 kt-loop
                        # streams behind the DMA
                        nc.sync.dma_start(out=wt[:, 0:1, :],
                                          in_=dr["w1"][j][:, 0:1, :])
                        nc.sync.dma_start(out=wt[:, 1:7, :],
                                          in_=dr["w1"][j][:, 1:7, :])
                        nc.sync.dma_start(out=wt[:, 7:, :],
                                          in_=dr["w1"][j][:, 7:, :])
                    elif cb == 0:
                        # the sweeps double the w1 rate (~300GB/s): the
                        # first tiles must beat the xt burst on sync, the
                        # rest spread across all three trigger queues
                        q = nc.sync if (sw == 0 and j < 8) else (
                            nc.sync, nc.gpsimd, nc.scalar)[j % 3]
                        q.dma_start(out=wt, in_=dr["w1"][j])
                    else:
                        nc.sync.dma_start(out=wt, in_=dr["w1"][j])
                    ps = pspool.tile([128, BCX], f32, tag="ps")
                    for kt in range(KD):
                        lhsT = wt[:, kt, :]
                        for b0 in range(s0, s0 + sn, N0):
                            nc.tensor.matmul(
                                ps[:, b0:b0 + N0], lhsT,
                                xt[kt][:, b0:b0 + N0],
                                start=(kt == 0), stop=(kt == KD - 1))
                    for h in range(s0, s0 + sn, N0):
                        nc.scalar.activation(a1[:, j, h:h + N0],
                                             ps[:, h:h + N0], AF.Sign,
                                             bias=c_sb[1][:, j:j + 1],
                                             scale=s_sb[1][:, j:j + 1])
                    # previous chunk's softmax tail goes here: its
                    # transposes land on the in-order PE queue behind 2
                    # j-tiles of L1 matmuls, by which time y4 is long
                    # ready -> no PE stall
                    if sw == 0 and j == 1 and pending_tail is not None:
                        emit_tail_softmax(pend_y4, pending_tail[1],
                                          pending_tail[2])
                        pending_tail = None
                    if cb == 0 and sw == 0 and j == 1:
                        # identity for the transposes: emitted once the
                        # startup DMA burst has drained
                        make_identity(nc, ident)
                    if cb == 0 and sw == 0 and j == 20:
                        # second xt half streams while sweep A finishes
                        load_xt(0, N0, BCX, t=xt0_t)

            # ---- layer 2: fp8 sign matmuls over the full chunk ----
            a2 = apool.tile([128, KH, BCX], fp8, tag="act")
            for j in range(JT):
                wt = wpool.tile([128, KH, 128], fp8, tag="w")
                nc.sync.dma_start(out=wt, in_=dr["w2"][j])
                ps = pspool.tile([128, BCX], f32, tag="ps")
                for kt in range(0, KH, 2):
                    lhsT = wt[:, kt:kt + 2, :]
                    for b0 in range(0, bc, N0):
                        nc.tensor.matmul(
                            ps[:, b0:b0 + N0], lhsT,
                            a1[:, kt:kt + 2, b0:b0 + N0],
                            start=(kt == 0), stop=(kt == KH - 2),
                            perf_mode=mybir.MatmulPerfMode.DoubleRow)
                for h in range(0, bc, N0):
                    nc.scalar.activation(a2[:, j, h:h + N0],
                                         ps[:, h:h + N0], AF.Sign,
                                         bias=c_sb[2][:, j:j + 1],
                                         scale=s_sb[2][:, j:j + 1])

            # ---- layer 3 + layer 4, in 512-col batch halves: each half's
            # L4 psum completes at the END of that half's j-loop, so half
            # h's softmax tail hides inside half h+1's (or the next
            # chunk's L1) matmul stream instead of dangling at the end.
            # w3 streams once per half (2x DMA, ~fp8 so cheap) ----
            a3 = apool.tile([128, KH, BCX], fp8, tag="act")
            for hb in range(bc // N0):
                b0 = hb * N0
                ps4 = ps4pool.tile([OP, N0], f32, tag="ps4")
                for j in range(JT):
                    wt = wpool.tile([128, KH, 128], fp8, tag="w")
                    nc.sync.dma_start(out=wt, in_=dr["w3"][j])
                    ps = pspool.tile([128, BCX], f32, tag="ps")
                    for kt in range(0, KH, 2):
                        nc.tensor.matmul(
                            ps[:, 0:N0], wt[:, kt:kt + 2, :],
                            a2[:, kt:kt + 2, b0:b0 + N0],
                            start=(kt == 0), stop=(kt == KH - 2),
                            perf_mode=mybir.MatmulPerfMode.DoubleRow)
                    nc.scalar.activation(a3[:, j, b0:b0 + N0],
                                         ps[:, 0:N0], AF.Sign,
                                         bias=c_sb[3][:, j:j + 1],
                                         scale=s_sb[3][:, j:j + 1])
                    # previous half's softmax tail: its transposes land on
                    # the in-order PE queue behind a j-tile of matmuls, by
                    # which time its y4 is long ready -> no PE stall
                    if j == 1 and pending_tail is not None:
                        emit_tail_softmax(pend_y4, pending_tail[1],
                                          pending_tail[2])
                        pending_tail = None
                    # L4 pair for (j-3, j-2): delayed two j-tiles so the
                    # in-order PE queue never stalls on the ACT epilogue
                    if j % 2 == 1 and j >= 3:
                        kt = j - 3
                        nc.tensor.matmul(
                            ps4[:, 0:N0], w4t[:, kt:kt + 2, :],
                            a3[:, kt:kt + 2, b0:b0 + N0],
                            start=(kt == 0), stop=False,
                            perf_mode=mybir.MatmulPerfMode.DoubleRow)
                nc.tensor.matmul(
                    ps4[:, 0:N0], w4t[:, KH - 2:KH, :],
                    a3[:, KH - 2:KH, b0:b0 + N0],
                    start=False, stop=True,
                    perf_mode=mybir.MatmulPerfMode.DoubleRow)
                pending_tail = (ps4, b_off + b0, N0)
                pend_y4 = emit_tail_y4(ps4, N0)
            b_off += bc

        # final half's tail (fully exposed -> pipeline it in two quarters)
        emit_tail_softmax(pend_y4, pending_tail[1], pending_tail[2], halves=2)

    # Pin every activation to the one table set that holds sign+exp+ln+
    # identity, so the ACT engine loads its table once instead of
    # ping-ponging 1.3us ACT_TABLE_LOADs between the Sign epilogues and
    # the softmax's Exp/Ln. Instance-bound override; the pass itself
    # (rust) picks any set containing the function, so we hide the
    # functions from every other set.
    import types
    import bass_rust as _br
    from concourse.hw_specs import get_activation_tables as _gat

    def _pinned_act_loads(self):
        keep = "natural_log_exp_and_others"
        strip = {AF.Sign, AF.Exp, AF.Ln, AF.Identity}
        tables = [(name, fns if name == keep else fns - strip)
                  for name, fns in _gat(self.m.arch).items()]
        has_act = any(isinstance(i, mybir.InstActivation)
                      for b in self.main_func.blocks for i in b.instructions)
        if has_act:
            _br.insert_act_table_loads(self, tables)

    nc.insert_act_table_loads = types.MethodType(_pinned_act_loads, nc)

    nc.compile()
    return nc


def _sign(w):
    return np.where(w >= 0, np.float32(1.0), np.float32(-1.0))


def _prep_inputs(inputs):
    """Host-side: binarize/fold/retile weights, transpose+split x per core."""
    f32 = np.float32
    w = {i: _sign(inputs[f"w{i}"].astype(f32)) for i in (1, 2, 3, 4)}

    # layer 1 weights: [hi;lo] share sign(w1).T stacked twice, pad to 13 tiles
    w1t = np.zeros((KD * 128, H), f32)
    w1t[:D_IN] = w[1].T
    w1t[D_IN:2 * D_IN] = w[1].T
    # NOTE: fp8 stationary x fp16 moving works and is exact for +-1 weights
    # but measured ~150ns/MM SLOWER on HW (mixed-dtype weight path) -- keep fp16
    w1b = np.ascontiguousarray(
        w1t.reshape(KD, 128, JT, 128).transpose(2, 1, 0, 3)).astype(np.float16)

    def hidden_w(wm):  # [4096, 4096] -> [jt, kp, kt, c] fp8
        return np.ascontiguousarray(
            wm.T.reshape(KH, 128, JT, 128).transpose(2, 1, 0, 3)
        ).astype(ml_dtypes.float8_e4m3)

    w2b, w3b = hidden_w(w[2]), hidden_w(w[3])

    # layer 4: [10, 4096] -> pad out to 16 -> [kp, kt, c] fp8
    w4t = np.zeros((H, OP), f32)
    w4t[:, :OUT] = w[4].T
    w4b = np.ascontiguousarray(
        w4t.reshape(KH, 128, OP).transpose(1, 0, 2)).astype(ml_dtypes.float8_e4m3)

    bnc = np.zeros((128, 3, 2, JT), f32)
    for l in (1, 2, 3):
        g = inputs[f"g{l}"].astype(f32)
        v = inputs[f"v{l}"].astype(f32)
        bb = inputs[f"b{l}"].astype(f32)
        m = inputs[f"m{l}"].astype(f32)
        be = inputs[f"be{l}"].astype(f32)
        s = g / np.sqrt(v + np.float32(BN_EPS))
        c = (bb - m) * s + be
        bnc[:, l - 1, 0, :] = s.reshape(JT, 128).T   # [128, JT]
        bnc[:, l - 1, 1, :] = c.reshape(JT, 128).T
    b4 = np.zeros((OP, 1), f32)
    b4[:OUT, 0] = inputs["b4"].astype(f32)

    shared = {"w1": w1b, "w2": w2b, "w3": w3b, "w4": w4b, "b4": b4,
              "bnc": bnc}

    x = inputs["x"].astype(f32)
    in_maps = []
    for c in range(N_CORES):
        xs = x[c * BS:(c + 1) * BS].T                     # [784, BS]
        hi = xs.astype(np.float16)
        lo = (xs - hi.astype(f32)).astype(np.float16)
        xp = np.zeros((KD * 128, BS), np.float16)
        xp[:D_IN] = hi
        xp[D_IN:2 * D_IN] = lo

        m = dict(shared)
        m["xt"] = np.ascontiguousarray(
            xp.reshape(KD, 128, BS).transpose(1, 0, 2))
        in_maps.append(m)
    return in_maps


def _run(inputs, trace=False):
    global _compiled
    from concourse.bass_utils import run_bass_kernel_spmd

    if _compiled is None:
        _compiled = _build_module()
    nc = _compiled
    in_maps = _prep_inputs(inputs)
    res = run_bass_kernel_spmd(nc, in_maps, core_ids=list(range(N_CORES)),
                               trace=trace)
    out = np.concatenate([res.results[c]["out"] for c in range(N_CORES)],
                         axis=0)
    return out.astype(np.float32), res


def kernel(**inputs):
    out, _ = _run(inputs, trace=False)
    return out



# revision 35
# speedup vs baseline: 1.0005x; 1.0005x over previous
"""Trainium2 Bass kernel for a 4-layer binarized MLP (BNN) in eval mode.

Network (B=16384, D_in=784, H=4096, D_out=10), all matmuls use sign(w):
  h1 = hardtanh(BN1(x @ sign(w1).T + b1))
  h2 = hardtanh(BN2(sign(h1) @ sign(w2).T + b2))
  h3 = hardtanh(BN3(sign(h2) @ sign(w3).T + b3))
  out = log_softmax(sign(h3) @ sign(w4).T + b4)

Key observations used here:
  * Only the SIGN of h1/h2/h3 matters downstream (hardtanh preserves sign),
    so each hidden layer reduces to  a_out = Sign(psum * s + c)  with
    s = g*rsqrt(v+eps), c = (b-m)*s + be  folded on the host.
  * sign() values are exactly representable in fp8e4/bf16, and matmuls of
    +-1 values accumulate exactly in fp32 PSUM -> layers 2..4 are exact.
  * Layer 1 needs ~fp32 precision on x: x is split into two fp16 terms
    (hi+lo captures ~22 mantissa bits; PE handles fp16 subnormals exactly).
    Both terms are CONCATENATED along K (with sign(w1).T stacked twice) so
    layer 1 is one [1664, B] x [1664, 4096] matmul accumulated in PSUM.
  * Data-parallel over 8 cores: batch is sharded 8 x 2048; weights are
    binarized+transposed+pre-tiled on the host and replicated.

Layout on device (feature-major activations; batch on the free dim):
  activations a_l : SBUF [128, 32 ktiles, B_CHUNK]   (fp8e4 +-1)
  weights W_l^T   : DRAM [32 jtiles, 128 kp, ktiles, 128 cols], streamed
                    per j-tile; matmul lhsT = wt[:, kt, :]  (stationary)
  psum            : [128, B_CHUNK] fp32, accumulated over ktiles
Final layer produces y4 [16, B] (10 valid rows), PE-transposed in 128-col
chunks into one PSUM tile, then log_softmax along the free dim and DMA to
the output [B_shard, 10].

Schedule notes (measured on HW; the kernel runs at ~97% PE stream
occupancy, which is the roofline for this decomposition -- fp16 streams 1
moving col/cycle and fp8 DoubleRow streams 2, so every 512-col matmul
instruction costs ~214ns regardless of layer; 5840 such instructions =
~1246us of irreducible stream time at the fp8 silicon peak):
  * PE warm-up: ~18 junk matmuls over a memset tile run while the first
    x/w DMAs stream, so the HAM clock-gate reaches 2.4 GHz before real
    work and the in-order PE queue never idles long enough to re-throttle.
  * Chunk 0's L1 runs in two 512-col sweeps so the startup only waits on
    half the xt bytes (the ramp is DMA-bandwidth-bound); the second xt
    half and sweep B's w1 re-stream hide inside sweep A.
  * L3+L4 run in 512-col batch halves; each half's L4 psum completes at
    that half's end, so every softmax tail hides inside the next half's
    (or next chunk's L1) matmul stream.  Only the last half-tail is
    exposed (~3.5us), pipelined in two quarters.
  * All activations are pinned to the single ACT table set that contains
    sign+exp+ln+identity ("natural_log_exp_and_others"); otherwise the
    hardware reloads 1.3us activation tables on every exp<->ln switch.
  * Softmax per half: 2 ACT ops (one EXP, one LN); per-group max/bias
    arithmetic runs on the otherwise-idle DVE via per-partition-scalar
    and broadcast ops; output DMAs rotate across the trigger queues.
  * Known dead ends (measured): fp8 stationary x fp16 moving runs ~2x
    slower per column (keep w1 fp16); 512-col chunks double the per-phase
    weight-DMA rate and starve the single weights queue; DoubleColumn/
    DoublePixel are uint8-only and matmul_mx is TRN3-only, so 2 MAC/cell/
    cycle (DoubleRow) is the TRN2 fp8 ceiling.
"""

import numpy as np
import ml_dtypes

# ---- problem constants (hardcoded per the harness contract) ----
B, D_IN, H, OUT = 16384, 784, 4096, 10
N_CORES = 8
BS = B // N_CORES          # 2048 rows per core
# batch chunks per core (chunk 0's L1 additionally runs in two 512-col
# sweeps so the DMA-bandwidth-bound ramp only waits on half the xt bytes)
CHUNKS = (1024, 1024)
BCX = max(CHUNKS)          # tile allocation size (chunks use a [:bc] slice)
KD = 13                    # 1664 = 13*128 k-tiles: [hi;lo] fp16 concat (2*784 padded)
KH = H // 128              # 32 k-tiles for hidden layers
JT = H // 128              # 32 output-channel tiles
N0 = 512                   # matmul moving free-dim chunk
OP = 16                    # padded output channels (10 -> 16)
BN_EPS = 1e-5

_BF16 = ml_dtypes.bfloat16

_compiled = None  # cache of (nc, run_fn)


def _build_module():
    import concourse.bass as bass
    import concourse.tile as tile
    from concourse import bacc, mybir
    from concourse.masks import make_identity
    from contextlib import ExitStack

    f32 = mybir.dt.float32
    bf16 = mybir.dt.bfloat16
    fp16 = mybir.dt.float16
    fp8 = mybir.dt.float8e4
    AF = mybir.ActivationFunctionType
    AX = mybir.AxisListType

    nc = bacc.Bacc("TRN2", target_bir_lowering=False, debug=False,
                   num_devices=N_CORES)

    dr = {}
    dr["xt"] = nc.dram_tensor("xt", [128, KD, BS], fp16, kind="ExternalInput").ap()
    dr["w1"] = nc.dram_tensor("w1", [JT, 128, KD, 128], fp16, kind="ExternalInput").ap()
    dr["w2"] = nc.dram_tensor("w2", [JT, 128, KH, 128], fp8, kind="ExternalInput").ap()
    dr["w3"] = nc.dram_tensor("w3", [JT, 128, KH, 128], fp8, kind="ExternalInput").ap()
    dr["w4"] = nc.dram_tensor("w4", [128, KH, OP], fp8, kind="ExternalInput").ap()
    # all six BN fold tensors packed into one DMA: [128, layer, {s,c}, JT]
    dr["bnc"] = nc.dram_tensor("bnc", [128, 3, 2, JT], f32, kind="ExternalInput").ap()
    dr["b4"] = nc.dram_tensor("b4", [OP, 1], f32, kind="ExternalInput").ap()
    out_dram = nc.dram_tensor("out", [BS, OUT], f32, kind="ExternalOutput").ap()

    with tile.TileContext(nc) as tc, ExitStack() as ctx:
        const = ctx.enter_context(tc.tile_pool(name="const", bufs=1))
        xt_pool = ctx.enter_context(tc.tile_pool(name="xt", bufs=1))
        # deep prefetch: several w1 j-tiles in flight during the ramp (a
        # late w1 DMA stalls its whole j-tile and lets the HAM clock-gate
        # re-throttle the PE)
        w1pool = ctx.enter_context(tc.tile_pool(name="w1p", bufs=4))
        wpool = ctx.enter_context(tc.tile_pool(name="wp", bufs=3))
        apool = ctx.enter_context(tc.tile_pool(name="acts", bufs=2))
        small = ctx.enter_context(tc.tile_pool(name="small", bufs=3))
        pspool = ctx.enter_context(tc.tile_pool(name="ps", bufs=2, space="PSUM"))
        ps4pool = ctx.enter_context(tc.tile_pool(name="ps4", bufs=1, space="PSUM"))
        pstpool = ctx.enter_context(tc.tile_pool(name="pst", bufs=2, space="PSUM"))

        # ---- PE warm-up: the HAM clock gate keeps the PE at 1.2 GHz until
        # it has been busy ~3.4us.  Real matmuls can't start until the first
        # x/w DMAs land (~2.5us after the engine barrier), so burn that idle
        # window on junk matmuls over a memset tile: the PE is then already
        # at 2.4 GHz when the real stream begins. ----
        junk = const.tile([128, 640], fp16, tag="junk")
        nc.gpsimd.memset(junk, 0.0)
        jps = pspool.tile([128, BCX], f32, tag="ps", name="warmps")
        for _ in range(18):
            nc.tensor.matmul(jps[:, 0:N0], junk[:, 0:128],
                             junk[:, 128:128 + N0], start=True, stop=True)

        def load_xt(b_off, c0, c1, t=None):
            # one SBUF tile, 2-ktile DMA slices alternating across the two
            # non-weight trigger queues (sync carries weights; only
            # gpsimd/sync/scalar can trigger DMAs).  Fewer descriptors =
            # more ramp bandwidth; slice-level deps keep the early matmuls
            # gated only on their own k-tiles.  kt0 ships alone so the
            # very first matmul starts as soon as possible.  [c0, c1) is
            # the column range loaded (the startup loads cols in halves,
            # passing the same tile handle so slice deps line up).
            qs = (nc.gpsimd, nc.scalar)
            if t is None:
                t = xt_pool.tile([128, KD, BCX], fp16, tag="xt", name="xtt")
            nc.gpsimd.dma_start(out=t[:, 0, c0:c1],
                                in_=dr["xt"][:, 0, b_off + c0:b_off + c1])
            for i, kt in enumerate(range(1, KD, 2)):
                ke = min(kt + 2, KD)
                qs[(i + 1) % 2].dma_start(
                    out=t[:, kt:ke, c0:c1],
                    in_=dr["xt"][:, kt:ke, b_off + c0:b_off + c1])
            return t

        # chunk 0's L1 runs in two 512-col sweeps, so only half the xt
        # bytes gate the ramp; the second half streams during sweep A
        xt0_t = load_xt(0, 0, N0)
        xt0 = [xt0_t[:, kt, :] for kt in range(KD)]

        # constants (single packed DMA; needed only by epilogues)
        bnc = const.tile([128, 3, 2, JT], f32, tag="bnc")
        nc.scalar.dma_start(out=bnc, in_=dr["bnc"])
        s_sb = {l: bnc[:, l - 1, 0, :] for l in (1, 2, 3)}
        c_sb = {l: bnc[:, l - 1, 1, :] for l in (1, 2, 3)}
        b4sb = const.tile([OP, 1], f32, tag="b4")
        nc.scalar.dma_start(out=b4sb, in_=dr["b4"])
        w4t = const.tile([128, KH, OP], fp8, tag="w4")
        nc.scalar.dma_start(out=w4t, in_=dr["w4"])
        # identity for the PE transposes: materialized lazily (first use is
        # the first chunk's softmax tail, deep into chunk 1's L1 j-loop) so
        # its gpsimd ops don't sit in the startup critical path
        ident = const.tile([OP, OP], f32, tag="ident")

        def emit_tail_y4(ps4_t, bc):
            """y4 = ps4 + b4, in halves on the idle DVE so it overlaps the
            last L3 epilogue (ACT) and transposes start sooner."""
            y4 = small.tile([OP, BCX], f32, tag="y4")
            for h in range(0, bc, bc // 2):
                nc.vector.tensor_scalar_add(y4[:, h:h + bc // 2],
                                            ps4_t[:, h:h + bc // 2],
                                            b4sb[:, 0:1])
            return y4

        def emit_tail_softmax(y4, b_off, bc, halves=1):
            """log_softmax with 2 ACT ops per half: one EXP, one LN.

            Per-group max-subtract / sums / final bias-add run on the idle
            DVE as per-partition-scalar ops, so the ACT engine never
            ping-pongs activation tables mid-chain.  halves=2 pipelines the
            chain in two batch halves (used for the final, fully exposed
            tail: half 1's ACT/DVE chain overlaps half 2's transposes).
            """
            from concourse.bass import broadcast_tensor_aps
            G = bc // 128
            Gh = G // halves
            GX = BCX // 128
            for h in range(halves):
                g0 = h * Gh
                pstf = pstpool.tile([128, GX, OP], f32, tag="pst", name="pstf")
                pst = pstf[:, 0:Gh, :]
                for t in range(Gh):
                    gt = g0 + t
                    nc.tensor.transpose(pst[:, t, :],
                                        y4[:, gt * 128:(gt + 1) * 128], ident)
                mx8 = small.tile([128, GX, 1], f32, tag="mx8", name="mx8")[:, 0:Gh, :]
                nc.vector.reduce_max(mx8[:, :, 0], pst[:, :, 0:OUT],
                                     axis=AX.X, negate=True)
                xs = small.tile([128, GX, OP], f32, tag="xs", name="xs")[:, 0:Gh, :]
                p_bc, m_bc = broadcast_tensor_aps(pst[:, :, :], mx8[:, :, :])
                nc.vector.tensor_add(xs, p_bc, m_bc)
                ex = small.tile([128, GX, OP], f32, tag="ex", name="ex")[:, 0:Gh, :]
                nc.scalar.activation(ex, xs, AF.Exp)
                sm8 = small.tile([128, GX, 1], f32, tag="sm8", name="sm8")[:, 0:Gh, :]
                nc.vector.reduce_sum(sm8[:, :, 0], ex[:, :, 0:OUT], axis=AX.X)
                lg8 = small.tile([128, GX, 1], f32, tag="lg8", name="lg8")[:, 0:Gh, :]
                nc.scalar.activation(lg8, sm8, AF.Ln)
                b28 = small.tile([128, GX, 1], f32, tag="b28", name="b28")[:, 0:Gh, :]
                nc.vector.tensor_sub(b28, mx8, lg8)
                oo = small.tile([128, GX, OP], f32, tag="oo", name="oo")[:, 0:Gh, :]
                p_bc2, b_bc = broadcast_tensor_aps(pst[:, :, :], b28[:, :, :])
                nc.vector.tensor_add(oo, p_bc2, b_bc)
                qs = (nc.sync, nc.gpsimd, nc.scalar)
                for t in range(Gh):
                    row0 = b_off + (g0 + t) * 128
                    qs[t % 3].dma_start(out=out_dram[row0:row0 + 128, :],
                                        in_=oo[:, t, 0:OUT])

        pending_tail = None  # (ps4_tile, b_off, bc) of the previous chunk

        b_off = 0
        for cb, bc in enumerate(CHUNKS):
            if cb == 0:
                xt = xt0
            else:
                xtt = load_xt(b_off, 0, bc)
                xt = [xtt[:, kt, :] for kt in range(KD)]

            # ---- layer 1: [hi;lo] fp16 concat matmul.  Chunk 0 runs in
            # two 512-col sweeps (w1 streamed once per sweep, spread over
            # the queues) so the ramp only waits on half the xt bytes ----
            a1 = apool.tile([128, KH, BCX], fp8, tag="act")
            sweeps = [(0, N0), (N0, N0)] if cb == 0 else [(0, bc)]
            for sw, (s0, sn) in enumerate(sweeps):
                for j in range(JT):
                    wt = w1pool.tile([128, KD, 128], fp16, tag="w1")
                    if cb == 0 and sw == 0 and j == 0:
                        # first k-tile lands first so matmul 0 starts
                        # sooner; the rest in two pieces so the j0 kt-loop
                        # streams behind the DMA
                        nc.sync.dma_start(out=wt[:, 0:1, :],
                                          in_=dr["w1"][j][:, 0:1, :])
                        nc.sync.dma_start(out=wt[:, 1:7, :],
                                          in_=dr["w1"][j][:, 1:7, :])
                        nc.sync.dma_start(out=wt[:, 7:, :],
                                          in_=dr["w1"][j][:, 7:, :])
                    elif cb == 0:
                        # the sweeps double the w1 rate (~300GB/s): the
                        # first tiles must beat the xt burst on sync, the
                        # rest spread across all three trigger queues
                        q = nc.sync if (sw == 0 and j < 8) else (
                            nc.sync, nc.gpsimd, nc.scalar)[j % 3]
                        q.dma_start(out=wt, in_=dr["w1"][j])
                    else:
                        nc.sync.dma_start(out=wt, in_=dr["w1"][j])
                    ps = pspool.tile([128, BCX], f32, tag="ps")
                    for kt in range(KD):
                        lhsT = wt[:, kt, :]
                        for b0 in range(s0, s0 + sn, N0):
                            nc.tensor.matmul(
                                ps[:, b0:b0 + N0], lhsT,
                                xt[kt][:, b0:b0 + N0],
                                start=(kt == 0), stop=(kt == KD - 1))
                    for h in range(s0, s0 + sn, N0):
                        nc.scalar.activation(a1[:, j, h:h + N0],
                                             ps[:, h:h + N0], AF.Sign,
                                             bias=c_sb[1][:, j:j + 1],
                                             scale=s_sb[1][:, j:j + 1])
                    # previous chunk's softmax tail goes here: its
                    # transposes land on the in-order PE queue behind 2
                    # j-tiles of L1 matmuls, by which time y4 is long
                    # ready -> no PE stall
                    if sw == 0 and j == 1 and pending_tail is not None:
                        emit_tail_softmax(pend_y4, pending_tail[1],
                                          pending_tail[2])
                        pending_tail = None
                    if cb == 0 and sw == 0 and j == 1:
                        # identity for the transposes: emitted once the
                        # startup DMA burst has drained
                        make_identity(nc, ident)
                    if cb == 0 and sw == 0 and j == 20:
                        # second xt half streams while sweep A finishes
                        load_xt(0, N0, BCX, t=xt0_t)

            # ---- layer 2: fp8 sign matmuls over the full chunk ----
            a2 = apool.tile([128, KH, BCX], fp8, tag="act")
            for j in range(JT):
                wt = wpool.tile([128, KH, 128], fp8, tag="w")
                nc.sync.dma_start(out=wt, in_=dr["w2"][j])
                ps = pspool.tile([128, BCX], f32, tag="ps")
                for kt in range(0, KH, 2):
                    lhsT = wt[:, kt:kt + 2, :]
                    for b0 in range(0, bc, N0):
                        nc.tensor.matmul(
                            ps[:, b0:b0 + N0], lhsT,
                            a1[:, kt:kt + 2, b0:b0 + N0],
                            start=(kt == 0), stop=(kt == KH - 2),
                            perf_mode=mybir.MatmulPerfMode.DoubleRow)
                for h in range(0, bc, N0):
                    nc.scalar.activation(a2[:, j, h:h + N0],
                                         ps[:, h:h + N0], AF.Sign,
                                         bias=c_sb[2][:, j:j + 1],
                                         scale=s_sb[2][:, j:j + 1])

            # ---- layer 3 + layer 4, in 512-col batch halves: each half's
            # L4 psum completes at the END of that half's j-loop, so half
            # h's softmax tail hides inside half h+1's (or the next
            # chunk's L1) matmul stream instead of dangling at the end.
            # w3 streams once per half (2x DMA, ~fp8 so cheap) ----
            a3 = apool.tile([128, KH, BCX], fp8, tag="act")
            for hb in range(bc // N0):
                b0 = hb * N0
                ps4 = ps4pool.tile([OP, N0], f32, tag="ps4")
                for j in range(JT):
                    wt = wpool.tile([128, KH, 128], fp8, tag="w")
                    nc.sync.dma_start(out=wt, in_=dr["w3"][j])
                    ps = pspool.tile([128, BCX], f32, tag="ps")
                    for kt in range(0, KH, 2):
                        nc.tensor.matmul(
                            ps[:, 0:N0], wt[:, kt:kt + 2, :],
                            a2[:, kt:kt + 2, b0:b0 + N0],
                            start=(kt == 0), stop=(kt == KH - 2),
                            perf_mode=mybir.MatmulPerfMode.DoubleRow)
                    nc.scalar.activation(a3[:, j, b0:b0 + N0],
                                         ps[:, 0:N0], AF.Sign,
                                         bias=c_sb[3][:, j:j + 1],
                                         scale=s_sb[3][:, j:j + 1])
                    # previous half's softmax tail: its transposes land on
                    # the in-order PE queue behind a j-tile of matmuls, by
                    # which time its y4 is long ready -> no PE stall
                    if j == 1 and pending_tail is not None:
                        emit_tail_softmax(pend_y4, pending_tail[1],
                                          pending_tail[2])
                        pending_tail = None
                    # L4 pair for (j-3, j-2): delayed two j-tiles so the
                    # in-order PE queue never stalls on the ACT epilogue
                    if j % 2 == 1 and j >= 3:
                        kt = j - 3
                        nc.tensor.matmul(
                            ps4[:, 0:N0], w4t[:, kt:kt + 2, :],
                            a3[:, kt:kt + 2, b0:b0 + N0],
                            start=(kt == 0), stop=False,
                            perf_mode=mybir.MatmulPerfMode.DoubleRow)
                nc.tensor.matmul(
                    ps4[:, 0:N0], w4t[:, KH - 2:KH, :],
                    a3[:, KH - 2:KH, b0:b0 + N0],
                    start=False, stop=True,
                    perf_mode=mybir.MatmulPerfMode.DoubleRow)
                pending_tail = (ps4, b_off + b0, N0)
                pend_y4 = emit_tail_y4(ps4, N0)
            b_off += bc

        # final half's tail (fully exposed -> pipeline it in two quarters)
        emit_tail_softmax(pend_y4, pending_tail[1], pending_tail[2], halves=2)

    # Pin every activation to the one table set that holds sign+exp+ln+
    # identity, so the ACT engine loads its table once instead of
    # ping-ponging 1.3us ACT_TABLE_LOADs between the Sign epilogues and
    # the softmax's Exp/Ln. Instance-bound override; the pass itself
    # (rust) picks any set containing the function, so we hide the
    # functions from every other set.
    import types
    import bass_rust as _br
    from concourse.hw_specs import get_activation_tables as _gat

    def _pinned_act_loads(self):
        keep = "natural_log_exp_and_others"
        strip = {AF.Sign, AF.Exp, AF.Ln, AF.Identity}
        tables = [(name, fns if name == keep else fns - strip)
                  for name, fns in _gat(self.m.arch).items()]
        has_act = any(isinstance(i, mybir.InstActivation)
                      for b in self.main_func.blocks for i in b.instructions)
        if has_act:
            _br.insert_act_table_loads(self, tables)

    nc.insert_act_table_loads = types.MethodType(_pinned_act_loads, nc)

    nc.compile()
    return nc


def _sign(w):
    return np.where(w >= 0, np.float32(1.0), np.float32(-1.0))


def _prep_inputs(inputs):
    """Host-side: binarize/fold/retile weights, transpose+split x per core."""
    f32 = np.float32
    w = {i: _sign(inputs[f"w{i}"].astype(f32)) for i in (1, 2, 3, 4)}

    # layer 1 weights: [hi;lo] share sign(w1).T stacked twice, pad to 13 tiles
    w1t = np.zeros((KD * 128, H), f32)
    w1t[:D_IN] = w[1].T
    w1t[D_IN:2 * D_IN] = w[1].T
    # NOTE: fp8 stationary x fp16 moving works and is exact for +-1 weights
    # but measured ~150ns/MM SLOWER on HW (mixed-dtype weight path) -- keep fp16
    w1b = np.ascontiguousarray(
        w1t.reshape(KD, 128, JT, 128).transpose(2, 1, 0, 3)).astype(np.float16)

    def hidden_w(wm):  # [4096, 4096] -> [jt, kp, kt, c] fp8
        return np.ascontiguousarray(
            wm.T.reshape(KH, 128, JT, 128).transpose(2, 1, 0, 3)
        ).astype(ml_dtypes.float8_e4m3)

    w2b, w3b = hidden_w(w[2]), hidden_w(w[3])

    # layer 4: [10, 4096] -> pad out to 16 -> [kp, kt, c] fp8
    w4t = np.zeros((H, OP), f32)
    w4t[:, :OUT] = w[4].T
    w4b = np.ascontiguousarray(
        w4t.reshape(KH, 128, OP).transpose(1, 0, 2)).astype(ml_dtypes.float8_e4m3)

    bnc = np.zeros((128, 3, 2, JT), f32)
    for l in (1, 2, 3):
        g = inputs[f"g{l}"].astype(f32)
        v = inputs[f"v{l}"].astype(f32)
        bb = inputs[f"b{l}"].astype(f32)
        m = inputs[f"m{l}"].astype(f32)
        be = inputs[f"be{l}"].astype(f32)
        s = g / np.sqrt(v + np.float32(BN_EPS))
        c = (bb - m) * s + be
        bnc[:, l - 1, 0, :] = s.reshape(JT, 128).T   # [128, JT]
        bnc[:, l - 1, 1, :] = c.reshape(JT, 128).T
    b4 = np.zeros((OP, 1), f32)
    b4[:OUT, 0] = inputs["b4"].astype(f32)

    shared = {"w1": w1b, "w2": w2b, "w3": w3b, "w4": w4b, "b4": b4,
              "bnc": bnc}

    x = inputs["x"].astype(f32)
    in_maps = []
    for c in range(N_CORES):
        xs = x[c * BS:(c + 1) * BS].T                     # [784, BS]
        hi = xs.astype(np.float16)
        lo = (xs - hi.astype(f32)).astype(np.float16)
        xp = np.zeros((KD * 128, BS), np.float16)
        xp[:D_IN] = hi
        xp[D_IN:2 * D_IN] = lo

        m = dict(shared)
        m["xt"] = np.ascontiguousarray(
            xp.reshape(KD, 128, BS).transpose(1, 0, 2))
        in_maps.append(m)
    return in_maps


def _run(inputs, trace=False):
    global _compiled
    from concourse.bass_utils import run_bass_kernel_spmd

    if _compiled is None:
        _compiled = _build_module()
    nc = _compiled
    in_maps = _prep_inputs(inputs)
    res = run_bass_kernel_spmd(nc, in_maps, core_ids=list(range(N_CORES)),
                               trace=trace)
    out = np.concatenate([res.results[c]["out"] for c in range(N_CORES)],
                         axis=0)
    return out.astype(np.float32), res


def kernel(**inputs):
    out, _ = _run(inputs, trace=False)
    return out

